# revision 1
# baseline (speedup 1.0000x reference)
"""DSA varlen sparse attention for Trainium2, 8 NeuronCores.

Strategy (token-sharded, K/V replicated per core):
  Per core c: tokens t in [c*256, (c+1)*256).
  Instead of gathering 64 K/V rows per token (536 MB of gather traffic),
  compute DENSE per-head scores S^T[j, t] = sum_d K[j,h,d] q[t,h,d] on the
  PE array in bf16, then multiply exp(S^T) by a scattered sparse weight
  matrix tsd^T[j, t] = sum_{k: topk_idx[t,k]=j} topk_scores[t,k]
  (zero elsewhere).  Because softmax's Z cancels in the reference's
  renormalization, the output is exactly
     out[t,h] = (sum_j exp(s[j,t]) * tsd[j,t] * V[j,h]) / (sum_j exp*tsd).
  The sparse scatter runs on-device with GPSIMD local_scatter
  (per-partition scatter, tokens on partitions); duplicate indices are
  pre-merged with a pairwise is_equal/reduce pass on the vector engine.
  The denominator rides as a leading "ones" column of V through the same
  PSUM accumulation.

  Engine schedule: DVE does the dedup chain while PE/ACT run per-head
  S^T matmuls + exp (which don't need the mask); GPSIMD scatters, PE
  transposes tsd, then phase B (mask-mul + AV matmuls + normalize)
  drains per (head, token-chunk).  bf16 inputs are prepared host-side
  (layout/sharding prep); all matmul accumulation is fp32 in PSUM.
"""

import numpy as np
import ml_dtypes
from contextlib import ExitStack

T, H, D, DV, TK = 2048, 8, 128, 128, 64
NCORES = 8
TC = T // NCORES          # 256 tokens per core
P = 128
TCH = TC // P             # 2 token chunks of 128
JC = T // P               # 16 key chunks of 128
SCALE = float(D) ** -0.5
HALF = 1024               # local_scatter num_elems limit is < 2048

_CACHE = {}
SAFE_DEDUP = False  # True: mark duplicate slots -1 (CoreSim asserts uniqueness)


def _build_program(safe_dedup=None):
    if safe_dedup is None:
        safe_dedup = SAFE_DEDUP
    import concourse.mybir as mybir
    import concourse.tile as tile
    from concourse import bacc

    dt = mybir.dt
    Alu = mybir.AluOpType
    Act = mybir.ActivationFunctionType
    Ax = mybir.AxisListType

    nc = bacc.Bacc(None, target_bir_lowering=False, debug=False)
    names = {}
    with ExitStack() as ctx:
        tc = ctx.enter_context(tile.TileContext(nc))
        dram = ctx.enter_context(tc.tile_pool(name="dram", bufs=1, space="DRAM"))
        sb = ctx.enter_context(tc.tile_pool(name="sb", bufs=1))
        pT_pool = ctx.enter_context(tc.tile_pool(name="pTp", bufs=8))
        sm = ctx.enter_context(tc.tile_pool(name="sm", bufs=1))
        sm2 = ctx.enter_context(tc.tile_pool(name="sm2", bufs=2))
        sps = ctx.enter_context(tc.tile_pool(name="spsum", bufs=2, space="PSUM"))
        ops = ctx.enter_context(tc.tile_pool(name="opsum", bufs=4, space="PSUM"))

        # ---------------- DRAM I/O (bf16 data prepped host-side) ----------
        q_d = dram.tile([P, H * TC], dt.bfloat16, kind="ExternalInput")
        k_d = dram.tile([P, H * T], dt.bfloat16, kind="ExternalInput")
        v_d = dram.tile([P, JC * H * (1 + DV)], dt.bfloat16, kind="ExternalInput")
        NSM = 2 * TCH * TK + P
        sm_d = dram.tile([P, NSM], dt.int16, kind="ExternalInput")
        out_d = dram.tile([P, TCH, H * DV], dt.float32, kind="ExternalOutput")

        names.update(
            q=q_d.name, k=k_d.name, v=v_d.name, sm=sm_d.name, out=out_d.name,
        )

        # ---------------- SBUF persistent ----------------
        kT = sb.tile([P, H, T], dt.bfloat16, tag="kT")                 # 32KB/p
        vE = sb.tile([P, JC, H, 1 + DV], dt.bfloat16, tag="vE")        # 33KB/p
        vE_half = [vE[:, 0 : JC // 2], vE[:, JC // 2 : JC]]
        qT = sb.tile([P, H, TC], dt.bfloat16, tag="qT")
        # low half padded by 2: clamp parks out-of-half indices at col 1024
        tsd = sb.tile([P, TCH, HALF + 2 + HALF], dt.bfloat16, tag="tsd")
        tsdT = sb.tile([P, JC, TC], dt.bfloat16, tag="tsdT")
        smalls = sb.tile([P, NSM], dt.int16, tag="smalls")
        idx16 = smalls[:, 0 : TCH * TK].rearrange("p (a b) -> p a b", a=TCH)
        tsbf = (
            smalls[:, TCH * TK : 2 * TCH * TK]
            .bitcast(dt.bfloat16).rearrange("p (a b) -> p a b", a=TCH)
        )
        ident = smalls[:, 2 * TCH * TK :].bitcast(dt.bfloat16)
        outs = sb.tile([P, TCH, H * DV], dt.float32, tag="outs")

        # ---------------- loads (small first; all HWDGE) ----------------
        # single HWDGE queue, FIFO = priority order (RR packet scheduling
        # otherwise makes everything complete together at the end)
        nc.sync.dma_start(out=smalls[:], in_=sm_d[:])
        nc.sync.dma_start(out=kT[:, 0, 0:512], in_=k_d[:, 0:512])
        nc.sync.dma_start(out=qT[:, 0, :], in_=q_d[:, 0:TC])
        nc.sync.dma_start(out=kT[:, 0, 512:T], in_=k_d[:, 512:T])
        nc.sync.dma_start(
            out=qT[:, 1:H, :].rearrange("p a b -> p (a b)"), in_=q_d[:, TC:]
        )
        HVB = JC // 2 * H * (1 + DV)
        for h in range(1, H):
            nc.sync.dma_start(
                out=kT[:, h, :], in_=k_d[:, h * T : (h + 1) * T]
            )
            if h == 4:
                nc.sync.dma_start(
                    out=vE_half[0].rearrange("p a b c -> p (a b c)"),
                    in_=v_d[:, 0:HVB],
                )
        nc.sync.dma_start(
            out=vE_half[1].rearrange("p a b c -> p (a b c)"), in_=v_d[:, HVB:],
        )

        # ---------------- dedup + scatter ----------------
        # Every slot of a duplicate group receives the same group-sum, so
        # scattering all slots is idempotent -- no last-occurrence masking
        # needed.  (CoreSim's local_scatter asserts uniqueness; HW only.)
        # Chunk 0 runs on DVE; chunk 1's big eq/mul ops run on the otherwise
        # idle GPSIMD (library reloads interleave with the scatters).
        assert not safe_dedup, "safe_dedup retired"

        def dedup_smalls(t, eng):
            b = sm2.tile([P, TK], dt.float32, tag=f"b{t}")
            eng.tensor_scalar_add(out=b[:], in0=idx16[:, t, :], scalar1=1.0)
            ilo = sm2.tile([P, TK], dt.int16, tag=f"ilo{t}")
            eng.tensor_scalar(
                out=ilo[:], in0=b[:], scalar1=float(HALF + 1), scalar2=-1.0,
                op0=Alu.min, op1=Alu.add,
            )
            hi1 = sm2.tile([P, TK], dt.float32, tag=f"hi1{t}")
            eng.tensor_scalar(
                out=hi1[:], in0=b[:], scalar1=-float(HALF), scalar2=0.0,
                op0=Alu.add, op1=Alu.max,
            )
            ihi = sm2.tile([P, TK], dt.int16, tag=f"ihi{t}")
            eng.tensor_scalar_add(out=ihi[:], in0=hi1[:], scalar1=-1.0)
            return ilo, ihi

        def scatter_chunk(t, tsum, ilo, ihi):
            nc.gpsimd.local_scatter(
                out_ap=tsd[:, t, 0 : HALF + 2], data_ap=tsum[:], idxs_ap=ilo[:],
                channels=P, num_elems=HALF + 2, num_idxs=TK,
            )
            nc.gpsimd.local_scatter(
                out_ap=tsd[:, t, HALF + 2 :], data_ap=tsum[:], idxs_ap=ihi[:],
                channels=P, num_elems=HALF, num_idxs=TK,
            )

        eq1 = sm.tile([P, TK, TK], dt.bfloat16, tag="eq1")
        # chunk 0 fully on DVE
        eq0 = sm.tile([P, TK, TK], dt.bfloat16, tag="eq0")
        nc.vector.tensor_tensor(
            out=eq0[:],
            in0=idx16[:, 0, :, None].to_broadcast([P, TK, TK]),
            in1=idx16[:, 0, None, :].to_broadcast([P, TK, TK]),
            op=Alu.is_equal,
        )
        nc.vector.tensor_tensor(
            out=eq0[:], in0=eq0[:],
            in1=tsbf[:, 0, None, :].to_broadcast([P, TK, TK]), op=Alu.mult,
        )
        tsum0 = sm2.tile([P, TK], dt.bfloat16, tag="tsum0")
        with nc.allow_low_precision("duplicate-group sums have <=4 terms"):
            nc.vector.tensor_reduce(out=tsum0[:], in_=eq0[:], axis=Ax.X, op=Alu.add)
        ilo0, ihi0 = dedup_smalls(0, nc.vector)
        scatter_chunk(0, tsum0, ilo0, ihi0)
        nc.vector.tensor_tensor(
            out=eq1[:],
            in0=idx16[:, 1, :, None].to_broadcast([P, TK, TK]),
            in1=idx16[:, 1, None, :].to_broadcast([P, TK, TK]),
            op=Alu.is_equal,
        )
        nc.vector.tensor_tensor(
            out=eq1[:], in0=eq1[:],
            in1=tsbf[:, 1, None, :].to_broadcast([P, TK, TK]), op=Alu.mult,
        )
        tsum1 = sm2.tile([P, TK], dt.bfloat16, tag="tsum1")
        with nc.allow_low_precision("duplicate-group sums have <=4 terms"):
            nc.vector.tensor_reduce(out=tsum1[:], in_=eq1[:], axis=Ax.X, op=Alu.add)
        ilo1, ihi1 = dedup_smalls(1, nc.vector)
        scatter_chunk(1, tsum1, ilo1, ihi1)

        # ------- phases A+B, manually interleaved per-engine streams -------
        # PE:  S^T h0,h1, TR0, S^T h2..h5 interleaved with AV-t0 h0.., TR1,
        #      S^T h6,h7, remaining AV-t0, AV-t1 h0..h7
        # ACT: exp h0 (drains-ch0 follow TR0), exp h1.., norms interleaved
        # DVE: dedup-ch0 (emitted above), mask-t0/h and dedup-ch1 pieces
        #      laddered, drains-ch1 after TR1, then mask-t1/h
        G = 4  # score chunks per PSUM tile (2 banks, double-buffered)

        def emit_st_head(h):
            pT = pT_pool.tile([P, JC, TC], dt.bfloat16, tag="pT")
            pTs.append(pT)
            for g in range(JC // G):
                sp = sps.tile([P, G, TC], dt.float32, tag="sp")
                for j in range(G):
                    jc = g * G + j
                    nc.tensor.matmul(
                        out=sp[:, j, :],
                        lhsT=kT[:, h, jc * P : (jc + 1) * P],
                        rhs=qT[:, h, :],
                        start=True, stop=True,
                    )
                nc.scalar.activation(
                    out=pT[:, g * G : (g + 1) * G, :], in_=sp[:],
                    func=Act.Exp, scale=SCALE,
                )

        def emit_tsd_transposes(t):
            # chunk-0 drains on ACT, chunk-1 on DVE
            for jc in range(JC):
                ps = ops.tile([P, P], dt.bfloat16, tag="op")
                off = jc * P if jc < JC // 2 else HALF + 2 + (jc - JC // 2) * P
                nc.tensor.transpose(
                    out=ps[:], in_=tsd[:, t, off : off + P], identity=ident[:]
                )
                dst = tsdT[:, jc, t * P : (t + 1) * P]
                if t == 0:
                    nc.scalar.copy(out=dst, in_=ps[:])
                else:
                    nc.vector.tensor_copy(out=dst, in_=ps[:])

        def emit_mask(h, t, eng=None):
            pT = pTs[h]
            (eng or nc.vector).tensor_tensor(
                out=pT[:, :, t * P : (t + 1) * P],
                in0=pT[:, :, t * P : (t + 1) * P],
                in1=tsdT[:, :, t * P : (t + 1) * P],
                op=Alu.mult,
            )

        def emit_av(h, t, act_norm=False):
            pT = pTs[h]
            op = ops.tile([P, 1 + DV], dt.float32, tag="op")
            for jc in range(JC):
                nc.tensor.matmul(
                    out=op[:],
                    lhsT=pT[:, jc, t * P : (t + 1) * P],
                    rhs=vE[:, jc, h, :],
                    start=(jc == 0), stop=(jc == JC - 1),
                )
            rec = sm2.tile([P, 1], dt.float32, tag="rec")
            nc.vector.reciprocal(out=rec[:], in_=op[:, 0:1])
            dst = outs[:, t, h * DV : (h + 1) * DV]
            if act_norm:
                nc.scalar.mul(out=dst, in_=op[:, 1 : 1 + DV], mul=rec[:])
            else:
                nc.vector.tensor_scalar(
                    out=dst, in0=op[:, 1 : 1 + DV],
                    scalar1=rec[:], scalar2=None, op0=Alu.mult,
                )

        pTs = []
        emit_st_head(0)
        emit_st_head(1)
        emit_st_head(2)
        emit_tsd_transposes(0)
        emit_st_head(3)
        emit_st_head(4)
        emit_st_head(5)
        emit_tsd_transposes(1)
        emit_st_head(6)
        emit_st_head(7)
        done_t1 = 0
        for h in range(H):
            emit_mask(h, 0)
            emit_av(h, 0, act_norm=(h >= 6))
            if h >= 2:
                emit_mask(done_t1, 1)
                emit_av(done_t1, 1)
                done_t1 += 1
        nc.sync.dma_start(out=out_d[:, 0, :], in_=outs[:, 0, :])
        for h in range(done_t1, H):
            emit_mask(h, 1)
            emit_av(h, 1, act_norm=True)
        nc.sync.dma_start(out=out_d[:, 0, :], in_=outs[:, 0, :])
        nc.sync.dma_start(out=out_d[:, 1, :], in_=outs[:, 1, :])

    nc.compile()
    return nc, names


def _get_program():
    key = ("prog", SAFE_DEDUP)
    if key not in _CACHE:
        _CACHE[key] = _build_program()
    return _CACHE[key]


def _host_inputs(q, k, v, idx, ts):
    """Build per-core in_maps (host-side shard/layout/dtype prep)."""
    bf16 = ml_dtypes.bfloat16
    identity = np.eye(P, dtype=np.float32).astype(bf16).view(np.int16)

    # kT[d, h, j] = K[j, h, d]  (device reads it as [P, H*T])
    k_full = np.ascontiguousarray(
        k.transpose(2, 1, 0).reshape(P, H * T)
    ).astype(bf16)
    # vE[p, jc, h, 0] = 1, vE[p, jc, h, 1:] = V[jc*128+p, h, :]
    v_r = v.reshape(JC, P, H, DV).transpose(1, 0, 2, 3)  # [P, JC, H, DV]
    v_full = np.ones((P, JC, H, 1 + DV), dtype=np.float32)
    v_full[:, :, :, 1:] = v_r
    v_full = v_full.reshape(P, JC * H * (1 + DV)).astype(bf16)

    maps = []
    for c in range(NCORES):
        sl = slice(c * TC, (c + 1) * TC)
        # qT[d, h, t] with t local to the shard
        qc = q[sl].transpose(2, 1, 0).reshape(P, H * TC)
        ic = idx[sl].astype(np.int16).reshape(TCH, P, TK).transpose(1, 0, 2)
        tc_ = ts[sl].reshape(TCH, P, TK).transpose(1, 0, 2).astype(bf16)
        packed = np.concatenate(
            [
                ic.reshape(P, TCH * TK),
                tc_.reshape(P, TCH * TK).view(np.int16),
                identity,
            ],
            axis=1,
        )
        maps.append(
            dict(
                q=np.ascontiguousarray(qc).astype(bf16),
                k=k_full,
                v=v_full,
                sm=np.ascontiguousarray(packed),
            )
        )
    return maps


def kernel(q_packed, k_packed, v_packed, topk_indices, topk_scores):
    from concourse.bass_utils import run_bass_kernel_spmd

    q = np.asarray(q_packed, dtype=np.float32)
    k = np.asarray(k_packed, dtype=np.float32)
    v = np.asarray(v_packed, dtype=np.float32)
    idx = np.asarray(topk_indices)
    ts = np.asarray(topk_scores, dtype=np.float32)

    nc, names = _get_program()
    logical_maps = _host_inputs(q, k, v, idx, ts)
    in_maps = [{names[key]: arr for key, arr in m.items()} for m in logical_maps]

    res = run_bass_kernel_spmd(nc, in_maps, core_ids=list(range(NCORES)))
    outn = names["out"]
    parts = []
    for c in range(NCORES):
        oc = res.results[c][outn]  # [P, TCH, H*DV]
        parts.append(oc.transpose(1, 0, 2).reshape(TC, H, DV))
    return np.concatenate(parts, axis=0).astype(np.float32)


if __name__ == "__main__":
    rng = np.random.default_rng(0)
    q = rng.standard_normal((T, H, D), dtype=np.float32)
    k = rng.standard_normal((T, H, D), dtype=np.float32)
    v = rng.standard_normal((T, H, DV), dtype=np.float32)
    idx = rng.integers(0, T, size=(T, TK), dtype=np.int64)
    ts = rng.random((T, TK), dtype=np.float32)
    out = kernel(q, k, v, idx, ts)
    print(out.shape, out.dtype)



# revision 4
# speedup vs baseline: 1.2183x; 1.2183x over previous
"""DSA varlen sparse attention for Trainium2, 8 NeuronCores — v4.

Token-sharded (256 tokens/core), K/V replicated, dense-S + sparse-mask
formulation (softmax Z cancels against the renormalization):
   out[t,h] = (sum_j exp(s[j,t]) * tsd[j,t] * V[j,h]) / (sum_j exp*tsd)

v4 structural changes vs the previous kernel:
  - Host pre-sorts each token's (topk_idx, topk_score) pairs by index
    (pure permutation; the output is invariant to per-token slot order).
    Duplicate-index merging then becomes a segmented suffix-sum scan
    over adjacent slots (O(K log K) on DVE, ~3us) instead of the O(K^2)
    all-pairs is_equal matrix (~22us).  Non-first slots of each run are
    parked out of range so local_scatter sees unique indices.
  - ACT runs ONLY the 32 exp instructions (normalize moved to GPSIMD,
    tsdT drains to DVE); ACT is the critical engine at ~33us busy.
  - V is loaded per-head (h-major DRAM layout) interleaved with K heads
    so AV(h) unblocks at ~10-23us instead of ~30us; AV matmuls and tsd
    transposes fill PE slack in the exp-paced S stream (which also keeps
    the PE p-state ramp warm).
  - Per-(h,t) output DMAs so the tail after the last exp is short.
"""

import numpy as np
import ml_dtypes
from contextlib import ExitStack

T, H, D, DV, TK = 2048, 8, 128, 128, 64
NCORES = 8
TC = T // NCORES          # 256 tokens per core
P = 128
TCH = TC // P             # 2 token chunks of 128
JC = T // P               # 16 key chunks of 128
SCALE = float(D) ** -0.5
HALF = 1024               # local_scatter num_elems limit is < 2048
G = 4                     # score jc-chunks per PSUM tile
NG = JC // G
NSM = 2 * TCH * TK + P

_CACHE = {}


def _build_program():
    import concourse.mybir as mybir
    import concourse.tile as tile
    from concourse import bacc

    dt = mybir.dt
    Alu = mybir.AluOpType
    Act = mybir.ActivationFunctionType

    nc = bacc.Bacc(None, target_bir_lowering=False, debug=False)
    names = {}
    with ExitStack() as ctx:
        tc = ctx.enter_context(tile.TileContext(nc))
        dram = ctx.enter_context(tc.tile_pool(name="dram", bufs=1, space="DRAM"))
        sb = ctx.enter_context(tc.tile_pool(name="sb", bufs=1))
        pT_pool = ctx.enter_context(tc.tile_pool(name="pTp", bufs=8))
        sm2 = ctx.enter_context(tc.tile_pool(name="sm2", bufs=2))
        sps = ctx.enter_context(tc.tile_pool(name="spsum", bufs=2, space="PSUM"))
        ops = ctx.enter_context(tc.tile_pool(name="opsum", bufs=4, space="PSUM"))

        # ---------------- DRAM I/O (bf16 data prepped host-side) ----------
        q_d = dram.tile([P, H * TC], dt.bfloat16, kind="ExternalInput")
        k_d = dram.tile([P, H * T], dt.bfloat16, kind="ExternalInput")
        # v is h-major so per-head loads are contiguous: [P, H, JC, 1+DV]
        v_d = dram.tile([P, H * JC * (1 + DV)], dt.bfloat16, kind="ExternalInput")
        sm_d = dram.tile([P, NSM], dt.int16, kind="ExternalInput")
        out_d = dram.tile([P, TCH, H * DV], dt.float32, kind="ExternalOutput")
        names.update(
            q=q_d.name, k=k_d.name, v=v_d.name, sm=sm_d.name, out=out_d.name,
        )

        # ---------------- SBUF persistent ----------------
        kT = sb.tile([P, H, T], dt.bfloat16, tag="kT")
        vE = sb.tile([P, H, JC, 1 + DV], dt.bfloat16, tag="vE")
        qT = sb.tile([P, H, TC], dt.bfloat16, tag="qT")
        tsd = sb.tile([P, TCH, 2 * (HALF + 2)], dt.bfloat16, tag="tsd")
        tsdT = sb.tile([P, JC, TC], dt.bfloat16, tag="tsdT")
        smalls = sb.tile([P, NSM], dt.int16, tag="smalls")
        idx16 = smalls[:, 0 : TCH * TK].rearrange("p (a b) -> p a b", a=TCH)
        tsbf = (
            smalls[:, TCH * TK : 2 * TCH * TK]
            .bitcast(dt.bfloat16).rearrange("p (a b) -> p a b", a=TCH)
        )
        ident = smalls[:, 2 * TCH * TK :].bitcast(dt.bfloat16)
        outs = sb.tile([P, TCH, H * DV], dt.float32, tag="outs")

        # ---------------- loads (single sync HWDGE queue; FIFO = priority)
        def kload(h, a, b):
            nc.sync.dma_start(
                out=kT[:, h, a:b], in_=k_d[:, h * T + a : h * T + b]
            )

        HVB = JC * (1 + DV)

        def vload(h):
            nc.sync.dma_start(
                out=vE[:, h].rearrange("p a b -> p (a b)"),
                in_=v_d[:, h * HVB : (h + 1) * HVB],
            )

        kload(0, 0, 512)
        nc.sync.dma_start(out=qT[:, 0, :], in_=q_d[:, 0:TC])
        nc.sync.dma_start(out=smalls[:], in_=sm_d[:])
        kload(0, 512, T)
        nc.sync.dma_start(
            out=qT[:, 1:H, :].rearrange("p a b -> p (a b)"), in_=q_d[:, TC:]
        )
        kload(1, 0, T)
        vload(0)
        vload(1)
        kload(2, 0, T)
        vload(2)
        kload(3, 0, T)
        vload(3)
        kload(4, 0, T)
        vload(4)
        kload(5, 0, T)
        vload(5)
        vload(6)
        kload(6, 0, T)
        vload(7)
        kload(7, 0, T)

        # ---------------- dedup: segmented suffix-sum over sorted slots ---
        # Host sorted each token's slots by index, so duplicate groups are
        # contiguous runs.  acc[k] accumulates the within-run suffix sum via
        # log2(TK) doubling steps; the first slot of each run ends up with
        # the full run sum.  Non-first slots (nf=1) are parked out of range
        # so the scatters see unique indices.
        acc = sm2.tile([P, TCH, TK], dt.bfloat16, tag="acc")
        nc.vector.tensor_copy(out=acc[:], in_=tsbf[:])
        tmp = sm2.tile([P, TCH, TK], dt.bfloat16, tag="tmp")
        same = sm2.tile([P, TCH, TK], dt.bfloat16, tag="same")
        with nc.allow_low_precision("duplicate-group sums have few terms"):
            s = 1
            while s < TK:
                w = TK - s
                nc.vector.tensor_tensor(
                    out=same[:, :, 0:w], in0=idx16[:, :, s:TK],
                    in1=idx16[:, :, 0:w], op=Alu.is_equal,
                )
                nc.vector.tensor_tensor(
                    out=tmp[:, :, 0:w], in0=same[:, :, 0:w],
                    in1=acc[:, :, s:TK], op=Alu.mult,
                )
                nc.vector.tensor_tensor(
                    out=acc[:, :, 0:w], in0=acc[:, :, 0:w],
                    in1=tmp[:, :, 0:w], op=Alu.add,
                )
                s *= 2
        # nf[k] = 1 if slot k continues a run (not the first occurrence)
        nf = sm2.tile([P, TCH, TK], dt.float32, tag="nf")
        nc.vector.tensor_scalar(
            out=nf[:, :, 0:1], in0=idx16[:, :, 0:1],
            scalar1=0.0, scalar2=None, op0=Alu.mult,
        )
        nc.vector.tensor_tensor(
            out=nf[:, :, 1:TK], in0=idx16[:, :, 1:TK],
            in1=idx16[:, :, 0 : TK - 1], op=Alu.is_equal,
        )
        # bm = (idx+1) + nf*8192  (parks duplicate slots out of range)
        nfbig = sm2.tile([P, TCH, TK], dt.float32, tag="nfbig")
        nc.vector.tensor_scalar(
            out=nfbig[:], in0=nf[:], scalar1=8192.0, scalar2=None, op0=Alu.mult,
        )
        bm = sm2.tile([P, TCH, TK], dt.float32, tag="bm")
        nc.vector.tensor_scalar_add(out=bm[:], in0=idx16[:], scalar1=1.0)
        nc.vector.tensor_tensor(out=bm[:], in0=bm[:], in1=nfbig[:], op=Alu.add)
        # ilo = min(bm, HALF+1) - 1             in [0 .. HALF]
        # ihi = min(max(bm-HALF, 0), HALF+1)-1  in [-1 .. HALF]
        ilo = sm2.tile([P, TCH, TK], dt.int16, tag="ilo")
        ihi = sm2.tile([P, TCH, TK], dt.int16, tag="ihi")
        b2 = sm2.tile([P, TCH, TK], dt.float32, tag="b2")
        nc.vector.tensor_scalar(
            out=ilo[:], in0=bm[:], scalar1=float(HALF + 1), scalar2=-1.0,
            op0=Alu.min, op1=Alu.add,
        )
        nc.vector.tensor_scalar(
            out=b2[:], in0=bm[:], scalar1=float(-HALF), scalar2=0.0,
            op0=Alu.add, op1=Alu.max,
        )
        nc.vector.tensor_scalar(
            out=ihi[:], in0=b2[:], scalar1=float(HALF + 1), scalar2=-1.0,
            op0=Alu.min, op1=Alu.add,
        )

        for t in range(TCH):
            nc.gpsimd.local_scatter(
                out_ap=tsd[:, t, 0 : HALF + 2], data_ap=acc[:, t],
                idxs_ap=ilo[:, t], channels=P, num_elems=HALF + 2, num_idxs=TK,
            )
            nc.gpsimd.local_scatter(
                out_ap=tsd[:, t, HALF + 2 : 2 * HALF + 4], data_ap=acc[:, t],
                idxs_ap=ihi[:, t], channels=P, num_elems=HALF + 2, num_idxs=TK,
            )

        # ------------------ per-head S^T / exp / mask / AV ----------------
        pTs = []
        tr_psums = {}
        extras = []        # thunks emitting PE work into exp-paced slack slots

        def emit_transpose(t, jc):
            ps = ops.tile([P, P], dt.bfloat16, tag="op")
            off = jc * P if jc < JC // 2 else HALF + 2 + (jc - JC // 2) * P
            nc.tensor.transpose(
                out=ps[:], in_=tsd[:, t, off : off + P], identity=ident[:]
            )
            tr_psums[(t, jc)] = ps

        def emit_drains(t):
            for jc in range(JC):
                ps = tr_psums.pop((t, jc))
                nc.vector.tensor_copy(
                    out=tsdT[:, jc, t * P : (t + 1) * P], in_=ps[:]
                )

        def emit_st_head(h, n_extra=0):
            pT = pT_pool.tile([P, JC, TC], dt.bfloat16, tag="pT")
            pTs.append(pT)
            for g in range(NG):
                sp = sps.tile([P, G, TC], dt.float32, tag="sp")
                for j in range(G):
                    jc = g * G + j
                    nc.tensor.matmul(
                        out=sp[:, j, :],
                        lhsT=kT[:, h, jc * P : (jc + 1) * P],
                        rhs=qT[:, h, :],
                        start=True, stop=True,
                    )
                nc.scalar.activation(
                    out=pT[:, g * G : (g + 1) * G, :], in_=sp[:],
                    func=Act.Exp, scale=SCALE,
                )
                for _ in range(n_extra):
                    if extras:
                        extras.pop(0)()

        def emit_mask(h, g, t):
            pT = pTs[h]
            nc.vector.tensor_tensor(
                out=pT[:, g * G : (g + 1) * G, t * P : (t + 1) * P],
                in0=pT[:, g * G : (g + 1) * G, t * P : (t + 1) * P],
                in1=tsdT[:, g * G : (g + 1) * G, t * P : (t + 1) * P],
                op=Alu.mult,
            )

        def emit_masks(h, ts_=None):
            for t in (range(TCH) if ts_ is None else ts_):
                for g in range(NG):
                    emit_mask(h, g, t)

        def emit_av(h, t):
            pT = pTs[h]
            op = ops.tile([P, 1 + DV], dt.float32, tag="op")
            for jc in range(JC):
                nc.tensor.matmul(
                    out=op[:],
                    lhsT=pT[:, jc, t * P : (t + 1) * P],
                    rhs=vE[:, h, jc, :],
                    start=(jc == 0), stop=(jc == JC - 1),
                )
            rec = sm2.tile([P, 1], dt.float32, tag="rec")
            nc.vector.reciprocal(out=rec[:], in_=op[:, 0:1])
            dst = outs[:, t, h * DV : (h + 1) * DV]
            nc.vector.tensor_scalar(
                out=dst, in0=op[:, 1 : 1 + DV],
                scalar1=rec[:], scalar2=None, op0=Alu.mult,
            )
            nc.sync.dma_start(out=out_d[:, t, h * DV : (h + 1) * DV], in_=dst)

        # transposes ride PE slack slots in heads 1-2 (after the scatters
        # land); AVs ride slots from head 3 on.
        emit_st_head(0)
        for jc in range(JC):
            extras.append(lambda jc=jc: emit_transpose(0, jc))
        emit_st_head(1, n_extra=4)
        for jc in range(JC):
            extras.append(lambda jc=jc: emit_transpose(1, jc))
        emit_st_head(2, n_extra=4)
        emit_drains(0)
        emit_masks(0, ts_=[0])
        emit_masks(1, ts_=[0])
        emit_drains(1)
        emit_masks(2, ts_=[0])
        emit_masks(0, ts_=[1])
        emit_masks(1, ts_=[1])
        emit_masks(2, ts_=[1])
        for h, t in [(0, 0), (0, 1), (1, 0), (1, 1), (2, 0), (2, 1)]:
            extras.append(lambda h=h, t=t: emit_av(h, t))
        for h in range(3, H):
            emit_st_head(h, n_extra=1)
            emit_masks(h)
            extras.append(lambda h=h: emit_av(h, 0))
            extras.append(lambda h=h: emit_av(h, 1))
        while extras:
            extras.pop(0)()

    nc.compile()
    return nc, names


def _get_program():
    if "prog" not in _CACHE:
        _CACHE["prog"] = _build_program()
    return _CACHE["prog"]


def _host_inputs(q, k, v, idx, ts):
    """Per-core in_maps (host-side shard/layout/dtype prep).

    Sorts each token's (index, score) slot pairs by index — a pure
    permutation (the reference output is invariant to slot order) that
    lets the device merge duplicates with an adjacent-slot scan.
    """
    bf16 = ml_dtypes.bfloat16
    identity = np.eye(P, dtype=np.float32).astype(bf16).view(np.int16)

    # kT[d, h, j] = K[j, h, d]
    k_full = np.ascontiguousarray(
        k.transpose(2, 1, 0).reshape(P, H * T)
    ).astype(bf16)
    # vE[p, h, jc, 0] = 1, vE[p, h, jc, 1:] = V[jc*128+p, h, :]
    v_r = v.reshape(JC, P, H, DV).transpose(1, 2, 0, 3)  # [P, H, JC, DV]
    v_full = np.ones((P, H, JC, 1 + DV), dtype=np.float32)
    v_full[:, :, :, 1:] = v_r
    v_full = v_full.reshape(P, H * JC * (1 + DV)).astype(bf16)

    idx = np.asarray(idx)
    order = np.argsort(idx, axis=1, kind="stable")
    idx_s = np.take_along_axis(idx, order, axis=1)
    ts_s = np.take_along_axis(np.asarray(ts), order, axis=1)

    maps = []
    for c in range(NCORES):
        sl = slice(c * TC, (c + 1) * TC)
        qc = q[sl].transpose(2, 1, 0).reshape(P, H * TC)
        ic = idx_s[sl].astype(np.int16).reshape(TCH, P, TK).transpose(1, 0, 2)
        tc_ = ts_s[sl].reshape(TCH, P, TK).transpose(1, 0, 2).astype(bf16)
        packed = np.concatenate(
            [
                ic.reshape(P, TCH * TK),
                tc_.reshape(P, TCH * TK).view(np.int16),
                identity,
            ],
            axis=1,
        )
        maps.append(
            dict(
                q=np.ascontiguousarray(qc).astype(bf16),
                k=k_full,
                v=v_full,
                sm=np.ascontiguousarray(packed),
            )
        )
    return maps


def kernel(q_packed, k_packed, v_packed, topk_indices, topk_scores):
    from concourse.bass_utils import run_bass_kernel_spmd

    q = np.asarray(q_packed, dtype=np.float32)
    k = np.asarray(k_packed, dtype=np.float32)
    v = np.asarray(v_packed, dtype=np.float32)
    idx = np.asarray(topk_indices)
    ts = np.asarray(topk_scores, dtype=np.float32)

    nc, names = _get_program()
    logical_maps = _host_inputs(q, k, v, idx, ts)
    in_maps = [{names[key]: arr for key, arr in m.items()} for m in logical_maps]

    res = run_bass_kernel_spmd(nc, in_maps, core_ids=list(range(NCORES)))
    outn = names["out"]
    parts = []
    for c in range(NCORES):
        oc = res.results[c][outn]  # [P, TCH, H*DV]
        parts.append(oc.transpose(1, 0, 2).reshape(TC, H, DV))
    return np.concatenate(parts, axis=0).astype(np.float32)


if __name__ == "__main__":
    rng = np.random.default_rng(0)
    q = rng.standard_normal((T, H, D), dtype=np.float32)
    k = rng.standard_normal((T, H, D), dtype=np.float32)
    v = rng.standard_normal((T, H, DV), dtype=np.float32)
    idx = rng.integers(0, T, size=(T, TK), dtype=np.int64)
    ts = rng.random((T, TK), dtype=np.float32)
    out = kernel(q, k, v, idx, ts)
    print(out.shape, out.dtype)
